# revision 15
# baseline (speedup 1.0000x reference)
"""CavityLoss Trainium2 kernel (nn_CavityLoss_43722767073667), v4.

Mathematical reduction of the reference, exact in fp32 (verified):

    loss = -mean( gt * [pred < c*] * ln(pred) ),  c* = f32(128/255)

History: v1 35.2us (fp32, STT pipeline) -> v2 29.8us (fp16 transport; trace
showed STT has no fast-mode uop, DVE 16us spine) -> v3 (min/max algebra on
fast ops, but tensor_scalar+accum lowers to TENSOR_SCALAR_CACHE_REDUCE which
runs 1x -> DVE reductions are all slow) -> v4: NO reductions on DVE at all.

Per-element algebra on the fp16 grid (c16 = fp16(c*) = 0.501953125,
rel err vs f32 threshold semantics ~2e-4, gate is 2e-2):

    g_enc = (1-gt)*c16          host-side lossless recode of binary gt
    w   = max(p, g_enc)         DVE tensor_tensor   2x_1p (0.5 cyc/elem)
    z   = min(w, c16)           DVE tensor_scalar   4x_2p (0.25)
    ind = [w < c16]             DVE tensor_scalar   4x_2p (0.25), bf16
    s   = Ln(z * (1/c16))       ACT 1x, accum_out -> row sums (the only
                                free reduction in the machine)
        = ln p - ln c16  on critical voxels (gt=1 and p<c16)
        = ln(1.0f) = 0   exactly, on all excluded voxels
    N1  = sum(ind)              PE: 54 accumulating ind[:,c:c+128]^T @ ones
                                matmuls into one PSUM [128,1] (PE is idle)
    loss = -(sum(s) + N1*ln(c16)) / N        host, f64

Engine budget per core (884736 elems): DMA 3.54 MB ~8.7us, DVE ~9.0us,
ACT ~7.5us, PE ~6us fully overlapped. DVE order is w,z,ind per tile so
ACT's Ln(t) (gated by z via s_z) starts two ops after the tile lands.

Scheduling: one packed pred|g_enc DMA + one sem per tile; every instruction
has exactly one wait (TRN2 limit; consecutive standalone wait_ge
instructions are used where two conditions gate one op). Final accumulators
([128, NT] Ln row sums + [128,1] PSUM counts copied by DVE) leave in one
[128, NT+1] f32 DMA; host reduces in f64.
"""

import numpy as np

import concourse.bacc as bacc
import concourse.mybir as mybir
from concourse.bass_utils import run_bass_kernel_spmd

D = 192
N_CORES = 8
P = 128
TOTAL = D * D * D              # 7_077_888
PER_CORE = TOTAL // N_CORES    # 884_736
FREE = PER_CORE // P           # 6_912
SIZES = [512, 1280, 1792, 1792, 1280, 256]
assert sum(SIZES) == FREE
assert all(s % 128 == 0 for s in SIZES)
NT = len(SIZES)
OFFS = np.concatenate([[0], np.cumsum(SIZES)]).tolist()

C_STAR = np.float32(128.0) / np.float32(255.0)
C16 = float(np.float16(C_STAR))                    # 0.501953125, fp16-exact
INV = float(np.float32(1.0) / np.float32(C16))     # f32(C16)*f32(INV) == 1.0f
LN_C16 = float(np.log(np.float64(C16)))

_CACHE = {}


def _build():
    nc = bacc.Bacc("TRN2", name="cavity_loss")
    f32 = mybir.dt.float32
    f16 = mybir.dt.float16
    bf16 = mybir.dt.bfloat16
    inp = nc.dram_tensor("inp", [P, 2 * FREE], f16, kind="ExternalInput")
    out = nc.dram_tensor("out", [P, NT + 1], f32, kind="ExternalOutput")

    mx = mybir.AluOpType.max
    mn = mybir.AluOpType.min
    lt = mybir.AluOpType.is_lt
    Ln = mybir.ActivationFunctionType.Ln

    in_sb = nc.alloc_sbuf_tensor("in_sb", [P, 2 * FREE], f16).ap()
    w_sb = nc.alloc_sbuf_tensor("w_sb", [P, FREE], f16).ap()
    z_sb = nc.alloc_sbuf_tensor("z_sb", [P, FREE], f16).ap()
    l_sb = nc.alloc_sbuf_tensor("l_sb", [P, FREE], f16).ap()
    ind_sb = nc.alloc_sbuf_tensor("ind_sb", [P, FREE], bf16).ap()
    # cols 0..NT-1: ACT Ln row sums; col NT: PE counts (copied from PSUM)
    acc = nc.alloc_sbuf_tensor("acc_sb", [P, NT + 1], f32).ap()
    psum_n = nc.alloc_psum_tensor("psum_n", [P, 1], f32).ap()

    s_in = [nc.alloc_semaphore(f"s_in{t}") for t in range(NT)]
    s_z = nc.alloc_semaphore("s_z")
    s_ind = nc.alloc_semaphore("s_ind")
    s_mm = nc.alloc_semaphore("s_mm")
    s_cp = nc.alloc_semaphore("s_cp")
    s_out = nc.alloc_semaphore("s_out")

    # packed layout: tile t occupies cols [2o, 2o+2s) of inp/in_sb,
    # pred in the first s cols, g_enc in the next s
    def pr(t):
        o, s = OFFS[t], SIZES[t]
        return in_sb[:, 2 * o : 2 * o + s]

    def gr(t):
        o, s = OFFS[t], SIZES[t]
        return in_sb[:, 2 * o + s : 2 * o + 2 * s]

    def sl(t):
        return slice(OFFS[t], OFFS[t + 1])

    # sync: stream the packed tiles in on one HWDGE ring
    for t in range(NT):
        o, s = OFFS[t], SIZES[t]
        nc.sync.dma_start(
            in_sb[:, 2 * o : 2 * o + 2 * s], inp[:, 2 * o : 2 * o + 2 * s]
        ).then_inc(s_in[t], 16)

    # scalar: dummy Ln pulls the ~2.7us ACT_TABLE_LOAD into the DMA window,
    # then per-tile masked-log with row-sum accumulation. ACT is the last
    # engine standing, so it also copies the PE counts out of PSUM (ScE sits
    # next to PSUM) and issues the output DMA itself (HWDGE via ACT) —
    # everything after the last Ln is ACT program order, no sem hops.
    dummy = nc.alloc_sbuf_tensor("dummy_sb", [P, 1], f32).ap()
    nc.scalar.activation(dummy[:], nc.const_aps.tensor(1.0, (P, 1)), Ln)
    for t in range(NT):
        nc.scalar.wait_ge(s_z, t + 1)
        act = nc.scalar.activation(
            l_sb[:, sl(t)], z_sb[:, sl(t)], Ln, scale=INV,
            accum_out=acc[:, t : t + 1],
        )
    # the then_inc lands on the auto-emitted ACTIVATION_READ_ACCUMULATOR, so
    # s_cp certifies the final accumulator column is in SBUF
    act.then_inc(s_cp, 1)
    nc.scalar.wait_ge(s_mm, 1)
    nc.scalar.copy(acc[:, NT : NT + 1], psum_n[:]).then_inc(s_cp, 1)
    # HWDGE dispatch is sequencer-side and NOT data-ordered with the engine
    # ops above — gate it on both final writes or the DMA races the copy
    nc.scalar.wait_ge(s_cp, 2)
    nc.scalar.dma_start(out[:], acc[:]).then_inc(s_out, 16)

    # vector: w, z per tile (the z chain gates ACT); ind ops for the last
    # tiles are deferred past the final z so they never delay it
    def w_op(t):
        nc.vector.wait_ge(s_in[t], 16)
        nc.vector.tensor_tensor(w_sb[:, sl(t)], pr(t), gr(t), mx)

    def z_op(t):
        nc.vector.tensor_scalar(
            z_sb[:, sl(t)], w_sb[:, sl(t)], C16, None, mn
        ).then_inc(s_z, 1)

    def i_op(t):
        nc.vector.tensor_scalar(
            ind_sb[:, sl(t)], w_sb[:, sl(t)], C16, None, lt
        ).then_inc(s_ind, 1)

    w_op(0); z_op(0)
    for t in range(1, NT - 2):
        w_op(t); z_op(t); i_op(t - 1)
    w_op(NT - 2); z_op(NT - 2)
    w_op(NT - 1); z_op(NT - 1)
    for t in range(NT - 3, NT):
        i_op(t)

    # tensor: count critical voxels — accumulate ind^T @ ones chunks in PSUM
    ones16 = nc.const_aps.tensor(1.0, (P, 1), bf16)
    n_chunks = FREE // 128
    ci = 0
    for t in range(NT):
        nc.tensor.wait_ge(s_ind, t + 1)
        o, s = OFFS[t], SIZES[t]
        for c in range(o, o + s, 128):
            mm = nc.tensor.matmul(
                psum_n[:], ind_sb[:, c : c + 128], ones16,
                start=(ci == 0), stop=(ci == n_chunks - 1),
            )
            ci += 1
    mm.then_inc(s_mm, 1)

    # finalize: ACT issued the [128, NT+1] f32 out DMA above; sync just waits
    nc.sync.wait_ge(s_out, 16)

    nc.compile()
    return nc


def _get_nc():
    if "nc" not in _CACHE:
        _CACHE["nc"] = _build()
    return _CACHE["nc"]


def _pack(pred, gt):
    p = np.ascontiguousarray(np.asarray(pred, dtype=np.float32)).reshape(-1)
    g = np.ascontiguousarray(np.asarray(gt, dtype=np.float32)).reshape(-1)
    assert p.size == TOTAL and g.size == TOTAL
    p16 = p.astype(np.float16).reshape(N_CORES, P, FREE)
    g16 = ((np.float32(1.0) - g) * np.float32(C16)).astype(np.float16)
    g16 = g16.reshape(N_CORES, P, FREE)
    packed = np.empty((N_CORES, P, 2 * FREE), np.float16)
    for t in range(NT):
        o, s = OFFS[t], SIZES[t]
        packed[:, :, 2 * o : 2 * o + s] = p16[:, :, o : o + s]
        packed[:, :, 2 * o + s : 2 * o + 2 * s] = g16[:, :, o : o + s]
    return packed


def run_spmd(pred, gt, **kw):
    """Shard, run on 8 cores; returns BassKernelResults (kw e.g. trace=True)."""
    packed = _pack(pred, gt)
    in_maps = [{"inp": packed[c]} for c in range(N_CORES)]
    return run_bass_kernel_spmd(
        _get_nc(), in_maps, core_ids=list(range(N_CORES)), **kw
    )


def kernel(pred, gt):
    res = run_spmd(pred, gt)
    loss_sum = 0.0
    for r in res.results:
        a = r["out"].astype(np.float64)
        loss_sum += a[:, :NT].sum() + a[:, NT].sum() * LN_C16
    return np.asarray(np.float32(-loss_sum / TOTAL))


# revision 16
# speedup vs baseline: 1.0228x; 1.0228x over previous
"""CavityLoss Trainium2 kernel (nn_CavityLoss_43722767073667), v4.

Mathematical reduction of the reference, exact in fp32 (verified):

    loss = -mean( gt * [pred < c*] * ln(pred) ),  c* = f32(128/255)

History: v1 35.2us (fp32, STT pipeline) -> v2 29.8us (fp16 transport; trace
showed STT has no fast-mode uop, DVE 16us spine) -> v3 (min/max algebra on
fast ops, but tensor_scalar+accum lowers to TENSOR_SCALAR_CACHE_REDUCE which
runs 1x -> DVE reductions are all slow) -> v4: NO reductions on DVE at all.

Per-element algebra on the fp16 grid (c16 = fp16(c*) = 0.501953125,
rel err vs f32 threshold semantics ~2e-4, gate is 2e-2):

    g_enc = (1-gt)*c16          host-side lossless recode of binary gt
    w   = max(p, g_enc)         DVE tensor_tensor   2x_1p (0.5 cyc/elem)
    z   = min(w, c16)           DVE tensor_scalar   4x_2p (0.25)
    ind = [w < c16]             DVE tensor_scalar   4x_2p (0.25), bf16
    s   = Ln(z * (1/c16))       ACT 1x, accum_out -> row sums (the only
                                free reduction in the machine)
        = ln p - ln c16  on critical voxels (gt=1 and p<c16)
        = ln(1.0f) = 0   exactly, on all excluded voxels
    N1  = sum(ind)              PE: 54 accumulating ind[:,c:c+128]^T @ ones
                                matmuls into one PSUM [128,1] (PE is idle)
    loss = -(sum(s) + N1*ln(c16)) / N        host, f64

Engine budget per core (884736 elems): DMA 3.54 MB ~8.7us, DVE ~9.0us,
ACT ~7.5us, PE ~6us fully overlapped. DVE order is w,z,ind per tile so
ACT's Ln(t) (gated by z via s_z) starts two ops after the tile lands.

Scheduling: one packed pred|g_enc DMA + one sem per tile; every instruction
has exactly one wait (TRN2 limit; consecutive standalone wait_ge
instructions are used where two conditions gate one op). Final accumulators
([128, NT] Ln row sums + [128,1] PSUM counts copied by DVE) leave in one
[128, NT+1] f32 DMA; host reduces in f64.
"""

import numpy as np

import concourse.bacc as bacc
import concourse.mybir as mybir
from concourse.bass_utils import run_bass_kernel_spmd

D = 192
N_CORES = 8
P = 128
TOTAL = D * D * D              # 7_077_888
PER_CORE = TOTAL // N_CORES    # 884_736
FREE = PER_CORE // P           # 6_912
SIZES = [512, 1280, 1792, 1792, 1280, 256]
assert sum(SIZES) == FREE
assert all(s % 128 == 0 for s in SIZES)
NT = len(SIZES)
OFFS = np.concatenate([[0], np.cumsum(SIZES)]).tolist()

C_STAR = np.float32(128.0) / np.float32(255.0)
C16 = float(np.float16(C_STAR))                    # 0.501953125, fp16-exact
INV = float(np.float32(1.0) / np.float32(C16))     # f32(C16)*f32(INV) == 1.0f
LN_C16 = float(np.log(np.float64(C16)))

_CACHE = {}


def _build():
    nc = bacc.Bacc("TRN2", name="cavity_loss")
    f32 = mybir.dt.float32
    f16 = mybir.dt.float16
    bf16 = mybir.dt.bfloat16
    inp = nc.dram_tensor("inp", [P, 2 * FREE], f16, kind="ExternalInput")
    out = nc.dram_tensor("out", [P, NT + 1], f32, kind="ExternalOutput")

    mx = mybir.AluOpType.max
    mn = mybir.AluOpType.min
    lt = mybir.AluOpType.is_lt
    Ln = mybir.ActivationFunctionType.Ln

    in_sb = nc.alloc_sbuf_tensor("in_sb", [P, 2 * FREE], f16).ap()
    w_sb = nc.alloc_sbuf_tensor("w_sb", [P, FREE], f16).ap()
    z_sb = nc.alloc_sbuf_tensor("z_sb", [P, FREE], f16).ap()
    l_sb = nc.alloc_sbuf_tensor("l_sb", [P, FREE], f16).ap()
    ind_sb = nc.alloc_sbuf_tensor("ind_sb", [P, FREE], bf16).ap()
    # cols 0..NT-1: ACT Ln row sums; col NT: PE counts (copied from PSUM)
    acc = nc.alloc_sbuf_tensor("acc_sb", [P, NT + 1], f32).ap()
    psum_n = nc.alloc_psum_tensor("psum_n", [P, 1], f32).ap()

    s_in = [nc.alloc_semaphore(f"s_in{t}") for t in range(NT)]
    s_z = nc.alloc_semaphore("s_z")
    s_ind = nc.alloc_semaphore("s_ind")
    s_mm = nc.alloc_semaphore("s_mm")
    s_cp = nc.alloc_semaphore("s_cp")
    s_out = nc.alloc_semaphore("s_out")

    # packed layout: tile t occupies cols [2o, 2o+2s) of inp/in_sb,
    # pred in the first s cols, g_enc in the next s
    def pr(t):
        o, s = OFFS[t], SIZES[t]
        return in_sb[:, 2 * o : 2 * o + s]

    def gr(t):
        o, s = OFFS[t], SIZES[t]
        return in_sb[:, 2 * o + s : 2 * o + 2 * s]

    def sl(t):
        return slice(OFFS[t], OFFS[t + 1])

    # tile 0 is dispatched from the scalar queue (its preamble slot opens
    # ~0.6us before sync's first dispatch — the stream end is pinned to
    # cumulative bytes over the shared HBM pipe, so an earlier first
    # transfer pulls the whole stream forward); sync streams the rest
    def dma_in(engine, t):
        o, s = OFFS[t], SIZES[t]
        engine.dma_start(
            in_sb[:, 2 * o : 2 * o + 2 * s], inp[:, 2 * o : 2 * o + 2 * s]
        ).then_inc(s_in[t], 16)

    dma_in(nc.scalar, 0)
    for t in range(1, NT):
        dma_in(nc.sync, t)

    # scalar: dummy Ln pulls the ~2.7us ACT_TABLE_LOAD into the DMA window,
    # then per-tile masked-log with row-sum accumulation. ACT is the last
    # engine standing, so it also copies the PE counts out of PSUM (ScE sits
    # next to PSUM) and issues the output DMA itself (HWDGE via ACT) —
    # everything after the last Ln is ACT program order, no sem hops.
    dummy = nc.alloc_sbuf_tensor("dummy_sb", [P, 1], f32).ap()
    nc.scalar.activation(dummy[:], nc.const_aps.tensor(1.0, (P, 1)), Ln)
    for t in range(NT):
        nc.scalar.wait_ge(s_z, t + 1)
        act = nc.scalar.activation(
            l_sb[:, sl(t)], z_sb[:, sl(t)], Ln, scale=INV,
            accum_out=acc[:, t : t + 1],
        )
    # the then_inc lands on the auto-emitted ACTIVATION_READ_ACCUMULATOR, so
    # s_cp certifies the final accumulator column is in SBUF
    act.then_inc(s_cp, 1)
    nc.scalar.wait_ge(s_mm, 1)
    nc.scalar.copy(acc[:, NT : NT + 1], psum_n[:]).then_inc(s_cp, 1)
    # HWDGE dispatch is sequencer-side and NOT data-ordered with the engine
    # ops above — gate it on both final writes or the DMA races the copy
    nc.scalar.wait_ge(s_cp, 2)
    nc.scalar.dma_start(out[:], acc[:]).then_inc(s_out, 16)

    # vector: w, z per tile (the z chain gates ACT); ind ops for the last
    # tiles are deferred past the final z so they never delay it
    def w_op(t):
        nc.vector.wait_ge(s_in[t], 16)
        nc.vector.tensor_tensor(w_sb[:, sl(t)], pr(t), gr(t), mx)

    def z_op(t):
        nc.vector.tensor_scalar(
            z_sb[:, sl(t)], w_sb[:, sl(t)], C16, None, mn
        ).then_inc(s_z, 1)

    def i_op(t):
        nc.vector.tensor_scalar(
            ind_sb[:, sl(t)], w_sb[:, sl(t)], C16, None, lt
        ).then_inc(s_ind, 1)

    w_op(0); z_op(0)
    for t in range(1, NT - 2):
        w_op(t); z_op(t); i_op(t - 1)
    w_op(NT - 2); z_op(NT - 2)
    w_op(NT - 1); z_op(NT - 1)
    for t in range(NT - 3, NT):
        i_op(t)

    # tensor: count critical voxels — accumulate ind^T @ ones chunks in PSUM
    ones16 = nc.const_aps.tensor(1.0, (P, 1), bf16)
    n_chunks = FREE // 128
    ci = 0
    for t in range(NT):
        nc.tensor.wait_ge(s_ind, t + 1)
        o, s = OFFS[t], SIZES[t]
        for c in range(o, o + s, 128):
            mm = nc.tensor.matmul(
                psum_n[:], ind_sb[:, c : c + 128], ones16,
                start=(ci == 0), stop=(ci == n_chunks - 1),
            )
            ci += 1
    mm.then_inc(s_mm, 1)

    # finalize: ACT issued the [128, NT+1] f32 out DMA above; sync just waits
    nc.sync.wait_ge(s_out, 16)

    nc.compile()
    return nc


def _get_nc():
    if "nc" not in _CACHE:
        _CACHE["nc"] = _build()
    return _CACHE["nc"]


def _pack(pred, gt):
    p = np.ascontiguousarray(np.asarray(pred, dtype=np.float32)).reshape(-1)
    g = np.ascontiguousarray(np.asarray(gt, dtype=np.float32)).reshape(-1)
    assert p.size == TOTAL and g.size == TOTAL
    p16 = p.astype(np.float16).reshape(N_CORES, P, FREE)
    g16 = ((np.float32(1.0) - g) * np.float32(C16)).astype(np.float16)
    g16 = g16.reshape(N_CORES, P, FREE)
    packed = np.empty((N_CORES, P, 2 * FREE), np.float16)
    for t in range(NT):
        o, s = OFFS[t], SIZES[t]
        packed[:, :, 2 * o : 2 * o + s] = p16[:, :, o : o + s]
        packed[:, :, 2 * o + s : 2 * o + 2 * s] = g16[:, :, o : o + s]
    return packed


def run_spmd(pred, gt, **kw):
    """Shard, run on 8 cores; returns BassKernelResults (kw e.g. trace=True)."""
    packed = _pack(pred, gt)
    in_maps = [{"inp": packed[c]} for c in range(N_CORES)]
    return run_bass_kernel_spmd(
        _get_nc(), in_maps, core_ids=list(range(N_CORES)), **kw
    )


def kernel(pred, gt):
    res = run_spmd(pred, gt)
    loss_sum = 0.0
    for r in res.results:
        a = r["out"].astype(np.float64)
        loss_sum += a[:, :NT].sum() + a[:, NT].sum() * LN_C16
    return np.asarray(np.float32(-loss_sum / TOTAL))


# revision 17
# speedup vs baseline: 1.0486x; 1.0252x over previous
"""CavityLoss Trainium2 kernel (nn_CavityLoss_43722767073667), v4.

Mathematical reduction of the reference, exact in fp32 (verified):

    loss = -mean( gt * [pred < c*] * ln(pred) ),  c* = f32(128/255)

History: v1 35.2us (fp32, STT pipeline) -> v2 29.8us (fp16 transport; trace
showed STT has no fast-mode uop, DVE 16us spine) -> v3 (min/max algebra on
fast ops, but tensor_scalar+accum lowers to TENSOR_SCALAR_CACHE_REDUCE which
runs 1x -> DVE reductions are all slow) -> v4: NO reductions on DVE at all.
(v5-v7 variants — deferred indicator ops, ACT-issued output DMA, scalar-
queue first dispatch — all measured slower or equal; the ACT-issued DMA
also needs explicit semaphore gating because HWDGE dispatch is sequencer-
side and races the engine ops writing the source buffer.)

Per-element algebra on the fp16 grid (c16 = fp16(c*) = 0.501953125,
rel err vs f32 threshold semantics ~2e-4, gate is 2e-2):

    g_enc = (1-gt)*c16          host-side lossless recode of binary gt
    w   = max(p, g_enc)         DVE tensor_tensor   2x_1p (0.5 cyc/elem)
    z   = min(w, c16)           DVE tensor_scalar   4x_2p (0.25)
    ind = [w < c16]             DVE tensor_scalar   4x_2p (0.25), bf16
    s   = Ln(z * (1/c16))       ACT 1x, accum_out -> row sums (the only
                                free reduction in the machine)
        = ln p - ln c16  on critical voxels (gt=1 and p<c16)
        = ln(1.0f) = 0   exactly, on all excluded voxels
    N1  = sum(ind)              PE: 54 accumulating ind[:,c:c+128]^T @ ones
                                matmuls into one PSUM [128,1] (PE is idle)
    loss = -(sum(s) + N1*ln(c16)) / N        host, f64

Engine budget per core (884736 elems): DMA 3.54 MB ~8.7us @ 360-427 GB/s
(run-to-run HBM variance), DVE ~9.4us, ACT ~9us, PE ~6us fully overlapped.
DVE order is w,z,ind per tile so ACT's Ln(t) (gated by z via s_z) starts
two ops after the tile lands; late tiles shrink so the serial
z->Ln->read->out tail after the last DMA byte stays short.

Scheduling: one packed pred|g_enc DMA + one sem per tile; every instruction
has exactly one wait (TRN2 limit). Final accumulators ([128, NT] Ln row
sums + [128,1] PSUM counts copied by DVE) leave in one [128, NT+1] f32 DMA;
host reduces in f64.
"""

import numpy as np

import concourse.bacc as bacc
import concourse.mybir as mybir
from concourse.bass_utils import run_bass_kernel_spmd

D = 192
N_CORES = 8
P = 128
TOTAL = D * D * D              # 7_077_888
PER_CORE = TOTAL // N_CORES    # 884_736
FREE = PER_CORE // P           # 6_912
SIZES = [512, 1792, 1792, 1664, 896, 256]
assert sum(SIZES) == FREE
assert all(s % 128 == 0 for s in SIZES)
NT = len(SIZES)
OFFS = np.concatenate([[0], np.cumsum(SIZES)]).tolist()

C_STAR = np.float32(128.0) / np.float32(255.0)
C16 = float(np.float16(C_STAR))                    # 0.501953125, fp16-exact
INV = float(np.float32(1.0) / np.float32(C16))     # f32(C16)*f32(INV) == 1.0f
LN_C16 = float(np.log(np.float64(C16)))

_CACHE = {}


def _build():
    nc = bacc.Bacc("TRN2", name="cavity_loss")
    f32 = mybir.dt.float32
    f16 = mybir.dt.float16
    bf16 = mybir.dt.bfloat16
    inp = nc.dram_tensor("inp", [P, 2 * FREE], f16, kind="ExternalInput")
    out = nc.dram_tensor("out", [P, NT + 1], f32, kind="ExternalOutput")

    mx = mybir.AluOpType.max
    mn = mybir.AluOpType.min
    lt = mybir.AluOpType.is_lt
    Ln = mybir.ActivationFunctionType.Ln

    in_sb = nc.alloc_sbuf_tensor("in_sb", [P, 2 * FREE], f16).ap()
    w_sb = nc.alloc_sbuf_tensor("w_sb", [P, FREE], f16).ap()
    z_sb = nc.alloc_sbuf_tensor("z_sb", [P, FREE], f16).ap()
    l_sb = nc.alloc_sbuf_tensor("l_sb", [P, FREE], f16).ap()
    ind_sb = nc.alloc_sbuf_tensor("ind_sb", [P, FREE], bf16).ap()
    # cols 0..NT-1: ACT Ln row sums; col NT: PE counts (copied from PSUM)
    acc = nc.alloc_sbuf_tensor("acc_sb", [P, NT + 1], f32).ap()
    psum_n = nc.alloc_psum_tensor("psum_n", [P, 1], f32).ap()

    s_in = [nc.alloc_semaphore(f"s_in{t}") for t in range(NT)]
    s_z = nc.alloc_semaphore("s_z")
    s_ind = nc.alloc_semaphore("s_ind")
    s_acc = nc.alloc_semaphore("s_acc")
    s_mm = nc.alloc_semaphore("s_mm")
    s_cnt = nc.alloc_semaphore("s_cnt")
    s_out = nc.alloc_semaphore("s_out")

    # packed layout: tile t occupies cols [2o, 2o+2s) of inp/in_sb,
    # pred in the first s cols, g_enc in the next s
    def pr(t):
        o, s = OFFS[t], SIZES[t]
        return in_sb[:, 2 * o : 2 * o + s]

    def gr(t):
        o, s = OFFS[t], SIZES[t]
        return in_sb[:, 2 * o + s : 2 * o + 2 * s]

    def sl(t):
        return slice(OFFS[t], OFFS[t + 1])

    # sync: stream the packed tiles in on one HWDGE ring
    for t in range(NT):
        o, s = OFFS[t], SIZES[t]
        nc.sync.dma_start(
            in_sb[:, 2 * o : 2 * o + 2 * s], inp[:, 2 * o : 2 * o + 2 * s]
        ).then_inc(s_in[t], 16)

    # scalar: dummy Ln pulls the ~2.7us ACT_TABLE_LOAD into the DMA window,
    # then per-tile masked-log with row-sum accumulation
    dummy = nc.alloc_sbuf_tensor("dummy_sb", [P, 1], f32).ap()
    nc.scalar.activation(dummy[:], nc.const_aps.tensor(1.0, (P, 1)), Ln)
    for t in range(NT):
        nc.scalar.wait_ge(s_z, t + 1)
        nc.scalar.activation(
            l_sb[:, sl(t)], z_sb[:, sl(t)], Ln, scale=INV,
            accum_out=acc[:, t : t + 1],
        ).then_inc(s_acc, 1)

    # vector: w, z, ind per tile — all plain fast-mode ops, no reductions
    for t in range(NT):
        nc.vector.wait_ge(s_in[t], 16)
        nc.vector.tensor_tensor(w_sb[:, sl(t)], pr(t), gr(t), mx)
        nc.vector.tensor_scalar(
            z_sb[:, sl(t)], w_sb[:, sl(t)], C16, None, mn
        ).then_inc(s_z, 1)
        nc.vector.tensor_scalar(
            ind_sb[:, sl(t)], w_sb[:, sl(t)], C16, None, lt
        ).then_inc(s_ind, 1)
    # after all tiles: copy the PE count column out of PSUM
    nc.vector.wait_ge(s_mm, 1)
    nc.vector.tensor_copy(acc[:, NT : NT + 1], psum_n[:]).then_inc(s_cnt, 1)

    # tensor: count critical voxels — accumulate ind^T @ ones chunks in PSUM
    ones16 = nc.const_aps.tensor(1.0, (P, 1), bf16)
    n_chunks = FREE // 128
    ci = 0
    mm = None
    for t in range(NT):
        nc.tensor.wait_ge(s_ind, t + 1)
        o, s = OFFS[t], SIZES[t]
        for c in range(o, o + s, 128):
            mm = nc.tensor.matmul(
                psum_n[:], ind_sb[:, c : c + 128], ones16,
                start=(ci == 0), stop=(ci == n_chunks - 1),
            )
            ci += 1
    mm.then_inc(s_mm, 1)

    # finalize: one contiguous [128, NT+1] f32 DMA; host reduces in f64
    nc.sync.wait_ge(s_acc, NT)
    nc.sync.wait_ge(s_cnt, 1)
    nc.sync.dma_start(out[:], acc[:]).then_inc(s_out, 16)
    nc.sync.wait_ge(s_out, 16)

    nc.compile()
    return nc


def _get_nc():
    if "nc" not in _CACHE:
        _CACHE["nc"] = _build()
    return _CACHE["nc"]


def _pack(pred, gt):
    p = np.ascontiguousarray(np.asarray(pred, dtype=np.float32)).reshape(-1)
    g = np.ascontiguousarray(np.asarray(gt, dtype=np.float32)).reshape(-1)
    assert p.size == TOTAL and g.size == TOTAL
    p16 = p.astype(np.float16).reshape(N_CORES, P, FREE)
    g16 = ((np.float32(1.0) - g) * np.float32(C16)).astype(np.float16)
    g16 = g16.reshape(N_CORES, P, FREE)
    packed = np.empty((N_CORES, P, 2 * FREE), np.float16)
    for t in range(NT):
        o, s = OFFS[t], SIZES[t]
        packed[:, :, 2 * o : 2 * o + s] = p16[:, :, o : o + s]
        packed[:, :, 2 * o + s : 2 * o + 2 * s] = g16[:, :, o : o + s]
    return packed


def run_spmd(pred, gt, **kw):
    """Shard, run on 8 cores; returns BassKernelResults (kw e.g. trace=True)."""
    packed = _pack(pred, gt)
    in_maps = [{"inp": packed[c]} for c in range(N_CORES)]
    return run_bass_kernel_spmd(
        _get_nc(), in_maps, core_ids=list(range(N_CORES)), **kw
    )


def kernel(pred, gt):
    res = run_spmd(pred, gt)
    loss_sum = 0.0
    for r in res.results:
        a = r["out"].astype(np.float64)
        loss_sum += a[:, :NT].sum() + a[:, NT].sum() * LN_C16
    return np.asarray(np.float32(-loss_sum / TOTAL))
